# revision 23
# baseline (speedup 1.0000x reference)
"""AEV (ANI-style radial gaussian basis + vector features) on Trainium2, 8 cores.

Reference computation per molecule b (N=256 atoms):
    r_ij[b,i,j,:] = coord[b,j] - coord[b,i]
    d_ij[b,i,j]   = |r_ij|  (rc on the diagonal)
    fc            = 0.5*cos(pi*d/rc) + 0.5   where d < rc else 0
    gs[b,i,j,k]   = exp(-eta*(d - s_k)^2) * fc
    gv[b,i,j,c,k] = gs[b,i,j,k] * r_ij_c / d

Sharding: data-parallel over B=16 molecules -> 2 molecules per core, no
cross-core communication.  Outputs dominate (260 MiB f32 total) so the kernel
is HBM-write bound; compute is spread over PE (pairwise differences via
broadcast matmuls), ACT (sqrt/sin/square/exp, phase-batched by activation
table set) and DVE (cutoff, reciprocal, gaussian*fc, outer products).

Per-core tile structure: 2 molecules x 2 i-blocks of 128 partitions; j is the
free dim.  gs/gv are produced in j-major/k-minor layout so the output DMAs
write 8-24 KiB contiguous per partition row.
"""

import math
from contextlib import ExitStack

import numpy as np

import concourse.bass as bass
import concourse.mybir as mybir
import concourse.tile as tile
from concourse import bacc
from concourse.bass_utils import run_bass_kernel_spmd

B, N, NSH = 16, 256, 16
RMIN, RC_DEFAULT = 0.8, 5.0
ETA_DEFAULT = float(np.float32((1.0 / ((RC_DEFAULT - RMIN) / NSH)) ** 2))
SHIFTS_DEFAULT = np.linspace(RMIN, RC_DEFAULT, NSH + 1, dtype=np.float32)[:NSH]
NCORES = 8
BPC = B // NCORES  # molecules per core

F32 = mybir.dt.float32
I32 = mybir.dt.int32
AOP = mybir.AluOpType
AF = mybir.ActivationFunctionType


def _body(ctx, tc, coord, d_out, gs_out, gv_out, eta, rc, shifts, level=8):
    nc = tc.nc
    sqeta = math.sqrt(eta)
    pi = math.pi

    singles = ctx.enter_context(tc.tile_pool(name="singles", bufs=1))
    molp = ctx.enter_context(tc.tile_pool(name="mol", bufs=2))
    sqp = ctx.enter_context(tc.tile_pool(name="sqp", bufs=2))
    smallp = ctx.enter_context(tc.tile_pool(name="small", bufs=4))
    fcp = ctx.enter_context(tc.tile_pool(name="fcp", bufs=2))
    t2p = ctx.enter_context(tc.tile_pool(name="t2", bufs=1))
    gp = ctx.enter_context(tc.tile_pool(name="g", bufs=2))
    gsp = ctx.enter_context(tc.tile_pool(name="gsp", bufs=3))
    gvp = ctx.enter_context(tc.tile_pool(name="gvp", bufs=2))
    psum = ctx.enter_context(tc.tile_pool(name="psum", bufs=2, space="PSUM"))

    # ---- constants ----
    # All matmul operands live at partition base 0 (PE rejects mixed bases).
    ones_s = singles.tile([1, N], F32)
    nc.vector.memset(ones_s[:], 1.0)
    neg_s = singles.tile([1, N], F32)
    nc.vector.memset(neg_s[:], -1.0)

    # per-partition bias columns for ACT ops (bias must be an AP unless 0/1)
    biases = singles.tile([128, 2 + NSH], F32)
    nc.vector.memset(biases[:, 1:2], math.log(0.5))
    for k in range(NSH):
        nc.vector.memset(biases[:, 2 + k : 3 + k], -sqeta * float(shifts[k]))
    b_exp = biases[:, 1:2]

    # rc^2 on the (i==j) diagonal of each i-block
    eyer = []
    for t in range(2):
        itmp = singles.tile([128, N], I32, tag=f"itmp{t}")
        e = singles.tile([128, N], F32, tag=f"eyer{t}")
        nc.gpsimd.iota(
            itmp[:], pattern=[[1, N]], base=-(128 * t), channel_multiplier=-1
        )
        nc.vector.tensor_scalar(e[:], itmp[:], 0, None, AOP.is_equal)
        nc.vector.tensor_scalar_mul(e[:], e[:], rc * rc)
        eyer.append(e)

    # ---- per-molecule: gather x/y/z component rows (partition 0 each) ----
    mol = []
    for m in range(BPC):
        cts = []
        for c in range(3):
            t_ = molp.tile([1, N], F32, tag=f"ct{c}")
            nc.sync.dma_start(t_[:], coord[m, :, c])
            cts.append(t_)
        mol.append(cts)

    tiles = [(m, t) for m in range(BPC) for t in range(2)]

    # ---- phase A/B: PE differences, sq, sqrt, recip, u ----
    # (ACT funcs here -- square, sqrt -- both live in sqrt_and_others)
    d_tiles, u_tiles = {}, {}
    for (m, t) in tiles:
        cts = mol[m]
        ps = psum.tile([128, 3 * N], F32, tag="ps")
        # r_c = xj - xi  ->  ps block c
        for c in range(3):
            blk = ps[:, c * N : (c + 1) * N]
            nc.tensor.matmul(
                blk, ones_s[0:1, 0:128], cts[c][:], start=True, stop=False
            )
            nc.tensor.matmul(
                blk, cts[c][0:1, bass.ts(t, 128)], neg_s[:], start=False, stop=True
            )
        # sq = dx^2 + dy^2 + dz^2  (+ rc^2 on the diagonal)
        sq3 = sqp.tile([128, 3 * N], F32, tag="sq3")
        nc.scalar.activation(sq3[:], ps[:, :], AF.Square)
        sqt = sqp.tile([128, N], F32, tag="sqt")
        if level >= 2:
            sq3v = sq3[:].rearrange("p (c j) -> p j c", c=3)
            nc.vector.tensor_reduce(
                sqt[:], sq3v, axis=mybir.AxisListType.X, op=AOP.add
            )
            nc.vector.scalar_tensor_tensor(
                sqt[:], sqt[:], 1.0, eyer[t][:], op0=AOP.mult, op1=AOP.add
            )
        else:
            nc.vector.tensor_copy(sqt[:], sq3[:, 0:N])
        dt_ = smallp.tile([128, N], F32, tag="d")
        nc.scalar.activation(dt_[:], sqt[:], AF.Sqrt)
        rec = smallp.tile([128, N], F32, tag="rec")
        uI = smallp.tile([128, 3 * N], F32, tag="u")  # [p, (j,c)] interleaved
        if level >= 3:
            nc.vector.reciprocal(rec[:], dt_[:])
            uv = uI[:].rearrange("p (j c) -> p j c", c=3)
            for c in range(3):
                nc.vector.tensor_tensor(
                    uv[:, :, c], ps[:, c * N : (c + 1) * N], rec[:], op=AOP.mult
                )
        nc.sync.dma_start(d_out[m, bass.ts(t, 128), :], dt_[:])
        d_tiles[(m, t)] = dt_
        u_tiles[(m, t)] = uI
    if level < 4:
        return

    # ---- phase C: half-angle sine  (trig_and_small; Sin valid on [-pi,pi]) ----
    # cos(pi*d/rc) + 1 = 2 - 2*sin^2(pi*d/(2rc)); clamp d to [0, rc] so the
    # sine argument stays in [0, pi/2] (out-of-cutoff pairs are masked later).
    s2_tiles = {}
    for (m, t) in tiles:
        dc = fcp.tile([128, N], F32, tag="dc")
        nc.vector.tensor_scalar_min(dc[:], d_tiles[(m, t)][:], rc)
        sv = fcp.tile([128, N], F32, tag="sv")
        nc.scalar.activation(sv[:], dc[:], AF.Sin, scale=pi / (2.0 * rc))
        s2 = smallp.tile([128, N], F32, tag="s2")
        nc.scalar.activation(s2[:], sv[:], AF.Square)
        s2_tiles[(m, t)] = s2
    if level < 5:
        return

    # ---- phase D-H: cutoff, gaussians, gs, gv  (exp_and_others) ----
    HJ = 128  # j-half size for the wide stages
    for (m, t) in tiles:
        dt_ = d_tiles[(m, t)]
        uI = u_tiles[(m, t)]
        mask = fcp.tile([128, N], F32, tag="mask")
        nc.vector.tensor_scalar(mask[:], dt_[:], rc, None, AOP.is_lt)
        # fc' = (cos + 1) * mask = (2 - 2 s^2) * mask ; 0.5 folded into exp bias
        ff = fcp.tile([128, N], F32, tag="ff")
        nc.vector.tensor_scalar(ff[:], s2_tiles[(m, t)][:], -2.0, 2.0, AOP.mult, AOP.add)
        fc = fcp.tile([128, N], F32, tag="fc")
        nc.vector.tensor_tensor(fc[:], ff[:], mask[:], op=AOP.mult)
        if level < 6:
            continue
        # T2[p, j, k] = eta*(d - s_k)^2, built per-k on ACT (Square in-set)
        t2 = t2p.tile([128, N * NSH], F32, tag="t2")
        t2v = t2[:].rearrange("p (j k) -> p j k", k=NSH)
        for k in range(NSH):
            nc.scalar.activation(
                t2v[:, :, k],
                dt_[:],
                AF.Square,
                bias=biases[:, 2 + k : 3 + k],
                scale=sqeta,
            )
        if level < 7:
            continue
        uv = uI[:].rearrange("p (j c) -> p j c", c=3)
        for h in range(2):
            seg = slice(h * HJ * NSH, (h + 1) * HJ * NSH)
            g = gp.tile([128, HJ * NSH], F32, tag="g")
            # 0.5 * exp(-eta*(d-s)^2)
            nc.scalar.activation(g[:], t2[:, seg], AF.Exp, bias=b_exp, scale=-1.0)
            gst = gsp.tile([128, HJ * NSH], F32, tag="gs")
            gss = gst[:].rearrange("p (j k) -> p j k", k=NSH)
            fcb = fc[:, bass.ts(h, HJ)].unsqueeze(2).broadcast_to((128, HJ, NSH))
            nc.vector.tensor_tensor(
                gss, g[:].rearrange("p (j k) -> p j k", k=NSH), fcb, op=AOP.mult
            )
            nc.sync.dma_start(gs_out[m, bass.ts(t, 128), bass.ts(h, HJ), :], gss)
            if level < 8:
                continue
            gvt = gvp.tile([128, HJ * 3 * NSH], F32, tag="gv")
            gvv = gvt[:].rearrange("p (j c k) -> p j c k", c=3, k=NSH)
            gsb = gss.unsqueeze(2).broadcast_to((128, HJ, 3, NSH))
            ub = uv[:, bass.ts(h, HJ), :].unsqueeze(3).broadcast_to((128, HJ, 3, NSH))
            nc.vector.tensor_tensor(gvv, gsb, ub, op=AOP.mult)
            nc.sync.dma_start(gv_out[m, bass.ts(t, 128), bass.ts(h, HJ), :, :], gvv)


def build_nc(eta=ETA_DEFAULT, rc=RC_DEFAULT, shifts=SHIFTS_DEFAULT, level=8) -> bass.Bass:
    nc = bacc.Bacc(
        "TRN2", target_bir_lowering=False, debug=False, num_devices=NCORES
    )
    coord = nc.declare_dram_parameter("coord", [BPC, N, 3], F32, isOutput=False)
    d_out = nc.declare_dram_parameter("d_ij", [BPC, N, N], F32, isOutput=True)
    gs_out = nc.declare_dram_parameter("gs", [BPC, N, N, NSH], F32, isOutput=True)
    gv_out = nc.declare_dram_parameter("gv", [BPC, N, N, 3, NSH], F32, isOutput=True)
    with tile.TileContext(nc) as tc, ExitStack() as ctx:
        _body(
            ctx, tc, coord, d_out, gs_out, gv_out,
            float(eta), float(rc), np.asarray(shifts, np.float32), level=level,
        )
    nc.compile()
    return nc


_NC_CACHE = {}


def _get_nc(eta, rc, shifts):
    key = (
        round(float(eta), 9),
        round(float(rc), 9),
        tuple(np.round(np.asarray(shifts, np.float64), 9)),
    )
    if key not in _NC_CACHE:
        _NC_CACHE[key] = build_nc(eta, rc, shifts)
    return _NC_CACHE[key]


def run(coord, shifts_s=None, eta_s=None, rc_s=None, trace=False, **trace_kwargs):
    coord = np.ascontiguousarray(np.asarray(coord, dtype=np.float32))
    assert coord.shape == (B, N, 3), coord.shape
    eta = float(eta_s) if eta_s is not None else ETA_DEFAULT
    rc = float(rc_s) if rc_s is not None else RC_DEFAULT
    shifts = (
        np.asarray(shifts_s, np.float32) if shifts_s is not None else SHIFTS_DEFAULT
    )
    nc = _get_nc(eta, rc, shifts)
    in_maps = [{"coord": coord[c * BPC : (c + 1) * BPC]} for c in range(NCORES)]
    out = run_bass_kernel_spmd(
        nc, in_maps, list(range(NCORES)), trace=trace, **trace_kwargs
    )
    res = out.results
    d = np.concatenate([r["d_ij"] for r in res], axis=0)
    gs = np.concatenate([r["gs"] for r in res], axis=0)
    gv = np.concatenate([r["gv"] for r in res], axis=0)
    return (d, gs, gv), out


def kernel(coord, shifts_s=None, eta_s=None, rc_s=None):
    (d, gs, gv), _ = run(coord, shifts_s=shifts_s, eta_s=eta_s, rc_s=rc_s)
    return d, gs, gv
